# revision 4
# baseline (speedup 1.0000x reference)
"""Trainium2 Bass kernel v3 for SAGAN-style self-attention.

Same fixed-point softmax math as v2 (validated at 5.5e-4), restructured for
real-hardware PE costs (ldweights ~ weight-columns/1.2GHz, ~60-cycle small-N
matmul floor, tile_position row/col concurrency):

  - Projections: kfg_f/kfg_g [C,34] weight blocks produce f^T/g^T replicated
    at partitions 0:34 and 64:98 via two concurrent column-tiled matmuls
    (shared x^T stream), one DVE/ACT copy per 512-slice.  fp16 everywhere on
    the S path (same PE speed as bf16, ~2x less quantization error).
  - QK: packs of 2 m-tiles as 2-way row-tiled concurrent matmuls (K=34 fits
    the 64-row group): lhsT/rhs at partitions 0:34 and 64:98, full-width
    [128,512] outputs into one 2-bank sg tile.
  - exp: ScalarE table-exp / DVE int16-bitcast convert per pack (validated
    BETA-matched split).
  - PV: orientation A (lhsT = [h|1] m-tile, 33 weight columns, N=512
    streams) with TWO query blocks column-tiled concurrently: O_pair
    [97, 512] PSUM bank holds qbA rows 0:33 and qbB rows 64:97; a K=1 zero
    matmul opens the bank so all PV matmuls accumulate with start=False.
  - Epilogue per pair: one [97,512] PSUM->SBUF bf16 copy; denominators
    extracted by 8 tiny PE transposes [1,128]->[128,1]; batched DVE
    reciprocal; output projection from Osb column slices (FWL weight loads);
    normalization + fp32 residual fused in one scalar_tensor_tensor per
    row-tile.
"""

import os
import sys

import numpy as np

try:
    import concourse.bass as bass  # noqa: F401
except Exception:  # pragma: no cover - path setup for fresh environments
    for _p in ("/opt/trn_rl_repo", "/root/.axon_site/_ro/trn_rl_repo"):
        if os.path.isdir(_p) and _p not in sys.path:
            sys.path.insert(0, _p)

B, H, W, C, D = 8, 64, 64, 256, 32
N_FULL = H * W  # 4096

LOG2E = 1.4426950408889634
LN2 = 0.6931471805599453
BETA = 0.7328702953376285  # ln2 + E[ln(1+f) - f*ln2], f~U[0,1) (DVE trick mean)

_BUILD_CACHE = {}
LAST_RESULTS = None
LAST_IN_MAPS = None

# engine for each pack's exp/convert within a query block (16 packs):
# 'A' = ScalarE exp, 'D' = DVE int16 convert
EXP_ENGINES = os.environ.get("K3_EXP_ENGINES", "ADADADADADADADAA")
QK_TILED = int(os.environ.get("K3_QK_TILED", "1"))
PV_PAIRED = int(os.environ.get("K3_PV_PAIRED", "1"))
EPI_TRANS = int(os.environ.get("K3_EPI_TRANS", "1"))
SG_BUFS = int(os.environ.get("K3_SG_BUFS", "3"))
USE_FP8 = int(os.environ.get("K3_FP8", "0"))
PT_BUFS = int(os.environ.get("K3_PT_BUFS", "40"))
K3_LAG = int(os.environ.get("K3_LAG", "4"))  # in pair-packs
PRO_PAIRS = int(os.environ.get("K3_PRO_PAIRS", "2"))


def _l2n64(v):
    return v / np.sqrt(np.maximum((v * v).sum(-1, keepdims=True), 1e-12))


def _sn_kernel_host(w, u):
    w64 = np.asarray(w, np.float64)
    u64 = np.asarray(u, np.float64)
    wr = w64.reshape(-1, w64.shape[-1])
    v = _l2n64(u64 @ wr.T)
    u2 = _l2n64(v @ wr)
    sigma = ((v @ wr) @ u2.T)[0, 0]
    return (w64 / sigma).astype(np.float32)


def _build(n, loop_k=1):
    import contextlib
    from collections import deque

    import concourse.bacc as bacc
    import concourse.mybir as mybir
    import concourse.tile as tile

    f32 = mybir.dt.float32
    f16 = mybir.dt.float16
    bf16 = mybir.dt.bfloat16
    i16 = mybir.dt.int16
    fp8 = mybir.dt.float8e4
    DR = mybir.MatmulPerfMode.DoubleRow
    EXP = mybir.ActivationFunctionType.Exp
    CPY = mybir.ActivationFunctionType.Copy
    AluOp = mybir.AluOpType

    NT = n // 128  # 128-row m-tiles
    NU = n // 512  # query blocks
    NP = NU // 2  # query-block pairs
    assert NU % 2 == 0 and NT % 2 == 0
    packs = [(2 * i, 2) for i in range(NT // 2)]
    last_pi = len(packs) - 1

    SCALE_ACT = LN2 / 128.0
    BIAS_ACT = BETA  # the +16384 c2 shift is added only on the DVE path

    nc = bacc.Bacc(
        "TRN2",
        target_bir_lowering=False,
        debug=False,
        enable_asserts=True,
        num_devices=8,
    )
    xb = nc.dram_tensor("xb", [n, C], f32, kind="ExternalInput").ap()
    xbh = nc.dram_tensor("xbh", [n, C], f16, kind="ExternalInput").ap()
    kfgf_d = nc.dram_tensor("kfgf", [C, 64], f16, kind="ExternalInput").ap()
    kfgg_d = nc.dram_tensor("kfgg", [C, 64], f16, kind="ExternalInput").ap()
    cbias_d = nc.dram_tensor("cbias", [128, 1], f32, kind="ExternalInput").ap()
    kh_d = nc.dram_tensor("kh", [C, D], f16, kind="ExternalInput").ap()
    kva_d = nc.dram_tensor("kva", [D + 1, C], bf16, kind="ExternalInput").ap()
    eye_d = nc.dram_tensor("eye", [128, 128], bf16, kind="ExternalInput").ap()
    y = nc.dram_tensor("y", [n, C], f32, kind="ExternalOutput").ap()

    with tile.TileContext(nc) as tc:
        with (
            tc.tile_pool(name="singles", bufs=1) as singles,
            tc.tile_pool(name="ptp", bufs=PT_BUFS) as ptp,
            tc.tile_pool(name="outp", bufs=4) as outp,
            tc.tile_pool(name="osbp", bufs=2) as osbp,
            tc.tile_pool(name="rcpp", bufs=2) as rcpp,
            tc.tile_pool(name="sgp", bufs=SG_BUFS, space="PSUM") as sgp,
            tc.tile_pool(name="opairp", bufs=1, space="PSUM") as opairp,
            tc.tile_pool(name="epip", bufs=1, space="PSUM") as epip,
        ):
            # ---------------- persistent SBUF tensors ----------------
            xrows = singles.tile([128, NT, C], f32)
            xT = singles.tile([128, 2, n], f16)
            fT = singles.tile([98, n], f16)  # rows 0:33 + replica at 64:97
            gT = singles.tile([98, n], f16)
            haug = singles.tile([128, NT * 33], bf16)  # [h | 1] per m-tile
            kfgf_sb = singles.tile([128, 2, 64], f16)
            kfgg_sb = singles.tile([128, 2, 64], f16)
            biasg_sb = singles.tile([128, 1], f32)
            ones128 = singles.tile([128, 1], bf16)
            kh_sb = singles.tile([128, 2, D], f16)
            kva_sb = singles.tile([97, C], bf16)  # kva at rows 0:33 and 64:97
            eye_sb = singles.tile([128, 128], bf16)
            zrow = singles.tile([1, 512], bf16)  # K=1 zero row (bank opener)
            bias_act = singles.tile([128, 1], f32)
            warm = singles.tile([1, 2], f32)

            nc.gpsimd.memset(warm, 0.0)
            nc.scalar.activation(
                out=warm[:, 1:2], in_=warm[:, 0:1], func=EXP, scale=1.0
            )

            xb_t = xb.rearrange("(t p) c -> p t c", p=128)

            def emit_xt(v):
                sl = slice(1024 * v, 1024 * (v + 1))
                for ch in range(2):
                    nc.sync.dma_start_transpose(
                        out=xT[:, ch, sl],
                        in_=xbh[sl, 128 * ch : 128 * (ch + 1)],
                    )

            emit_xt(0)
            nc.sync.dma_start(
                out=kfgf_sb, in_=kfgf_d.rearrange("(ch p) d -> p ch d", p=128)
            )
            nc.sync.dma_start(
                out=kfgg_sb, in_=kfgg_d.rearrange("(ch p) d -> p ch d", p=128)
            )
            nc.sync.dma_start(
                out=kh_sb, in_=kh_d.rearrange("(ch p) d -> p ch d", p=128)
            )
            nc.sync.dma_start(out=biasg_sb, in_=cbias_d)
            nc.gpsimd.memset(ones128, 1.0)
            nc.gpsimd.memset(
                haug.rearrange("p (k e) -> p k e", e=33)[:, :, 32:33], 1.0
            )
            nc.gpsimd.memset(zrow, 0.0)
            nc.gpsimd.memset(bias_act, BIAS_ACT)

            # ---------------- emission helpers ----------------
            O_tiles = {}

            def emit_chunk(v):
                """x^T, f^T, g^T (replicated), h rows for 1024-wide chunk v."""
                sl = slice(1024 * v, 1024 * (v + 1))
                if v > 0:
                    emit_xt(v)
                for h2 in range(2):
                    s5 = slice(1024 * v + 512 * h2, 1024 * v + 512 * (h2 + 1))
                    for w_sb, dst, on_act in (
                        (kfgf_sb, fT, True),
                        (kfgg_sb, gT, False),
                    ):
                        psR = epip.tile([128, 512], f32, tag="epi", name="psR")
                        for rep in range(2):
                            for ch in range(2):
                                nc.tensor.matmul(
                                    out=psR[64 * rep : 64 * rep + 64, :],
                                    lhsT=w_sb[:, ch, :],
                                    rhs=xT[:, ch, s5],
                                    start=(ch == 0),
                                    stop=(ch == 1),
                                    tile_position=(0, 64 * rep),
                                )
                        if on_act:
                            nc.scalar.activation(
                                out=dst[:, s5], in_=psR[0:98, :], func=CPY
                            )
                        else:
                            # g's const-128 rows (32, 96) via the bias; the
                            # psR rows there are zero weight columns
                            nc.vector.tensor_scalar_add(
                                out=dst[:, s5],
                                in0=psR[0:98, :],
                                scalar1=biasg_sb[0:98, :],
                            )
                hp = epip.tile([128, 8 * D], f32, tag="epi", name="hp")
                for k in range(8):
                    t = 8 * v + k
                    for ch in range(2):
                        nc.tensor.matmul(
                            out=hp[:, D * k : D * (k + 1)],
                            lhsT=xT[:, ch, 128 * t : 128 * (t + 1)],
                            rhs=kh_sb[:, ch, :],
                            start=(ch == 0),
                            stop=(ch == 1),
                        )
                nc.scalar.activation(
                    out=haug[:, 33 * 8 * v : 33 * 8 * (v + 1)].rearrange(
                        "p (k e) -> p k e", k=8
                    )[:, :, 0:D],
                    in_=hp.rearrange("p (k d) -> p k d", k=8),
                    func=CPY,
                )

            def emit_qk_exp(qb, pi):
                """2-way row-tiled t-tiles for pack pi of qb + exp/convert."""
                p0, sz = packs[pi]
                qs = slice(512 * qb, 512 * (qb + 1))
                sg = sgp.tile([128, 512 * sz], f32, tag="sg", name="sg")
                for i in range(sz):
                    mt = p0 + i
                    ro = 64 * i
                    nc.tensor.matmul(
                        out=sg[:, 512 * i : 512 * (i + 1)],
                        lhsT=fT[ro : ro + 33, 128 * mt : 128 * (mt + 1)],
                        rhs=gT[ro : ro + 33, qs],
                        start=True,
                        stop=True,
                    )
                pt = ptp.tile([128, 512 * sz], bf16, tag="pt", name="pt")
                if EXP_ENGINES[pi % len(EXP_ENGINES)] == "A":
                    nc.scalar.activation(
                        out=pt, in_=sg, func=EXP, scale=SCALE_ACT, bias=bias_act
                    )
                else:
                    nc.vector.tensor_scalar_add(
                        out=pt.bitcast(i16), in0=sg, scalar1=16384.0
                    )
                return pt

            def emit_pv(pair, pi, ptA, ptB):
                p0, sz = packs[pi]
                opair = O_tiles[pair]
                if pi == 0:
                    nc.tensor.matmul(
                        out=opair,
                        lhsT=zrow[:, 0:97],
                        rhs=zrow[:, 0:512],
                        start=True,
                        stop=False,
                        skip_group_check=True,
                    )
                for i in range(sz):
                    mt = p0 + i
                    lhs = haug[:, 33 * mt : 33 * mt + 33]
                    last = pi == last_pi and i == sz - 1
                    nc.tensor.matmul(
                        out=opair[0:33, :],
                        lhsT=lhs,
                        rhs=ptA[:, 512 * i : 512 * (i + 1)],
                        start=False,
                        stop=last,
                        skip_group_check=True,
                    )
                    if PV_PAIRED:
                        nc.tensor.matmul(
                            out=opair[64:97, :],
                            lhsT=lhs,
                            rhs=ptB[:, 512 * i : 512 * (i + 1)],
                            start=False,
                            stop=last,
                            skip_group_check=True,
                        )

            epi_q = deque()  # deferred epilogue pieces, drained 1 per pop

            def emit_epilogue(pair):
                # osb copy emitted immediately: it is the only reader of the
                # opair accumulator, so the next pair's opener (pool WAR)
                # must order after it
                opair = O_tiles[pair]
                osb = osbp.tile([97, 512], bf16, name="osb")
                nc.vector.tensor_copy(out=osb, in_=opair)
                rcp = rcpp.tile([128, 8], f32, name="rcp")

                def head():
                    svec = epip.tile([128, 4, 98], bf16, tag="epi", name="svec")
                    for jj in range(4):
                        nc.tensor.transpose(
                            out=svec[:, jj, 0:97],
                            in_=osb[:, 128 * jj : 128 * (jj + 1)],
                            identity=eye_sb[0:97, 0:97],
                        )
                    nc.vector.reciprocal(
                        out=rcp[:, 0:4],
                        in_=svec[:, :, 32:33].rearrange("p j e -> p (j e)"),
                    )
                    nc.vector.reciprocal(
                        out=rcp[:, 4:8],
                        in_=svec[:, :, 96:97].rearrange("p j e -> p (j e)"),
                    )

                def tile_j(j):
                    def go():
                        qb = 2 * pair + (j // 4)
                        jj = j % 4
                        nt = 4 * qb + jj
                        base = 0 if j < 4 else 64
                        op_ps = epip.tile([128, C], f32, tag="epi", name="op_ps")
                        nc.tensor.matmul(
                            out=op_ps,
                            lhsT=osb[base : base + 33, 128 * jj : 128 * (jj + 1)],
                            rhs=kva_sb[base : base + 33, :],
                            start=True,
                            stop=True,
                        )
                        ot = outp.tile([128, C], f32, name="ot")
                        nc.vector.scalar_tensor_tensor(
                            out=ot,
                            in0=op_ps,
                            scalar=rcp[:, j : j + 1],
                            in1=xrows[:, nt, :],
                            op0=AluOp.mult,
                            op1=AluOp.add,
                        )
                        nc.sync.dma_start(
                            out=y[128 * nt : 128 * (nt + 1), :], in_=ot
                        )

                    return go

                epi_q.append(head)
                for j in range(8):
                    epi_q.append(tile_j(j))

            # ---------------- emission schedule ----------------
            loop_cm = (
                tc.For_i(0, loop_k, 1, name="rep")
                if loop_k > 1
                else contextlib.nullcontext()
            )
            loop_cm.__enter__()

            pend = {}
            state = {"outstanding": 0, "cur": 0}

            def push_pair_pack(pair, pi):
                ptA = emit_qk_exp(2 * pair, pi)
                ptB = emit_qk_exp(2 * pair + 1, pi)
                pend.setdefault(pair, deque()).append((pi, ptA, ptB))
                state["outstanding"] += 1

            def pop_pv(force=False):
                while state["outstanding"] > (0 if force else K3_LAG):
                    if epi_q:
                        # emit one epilogue piece AND still pop a pack this
                        # turn, so PV never falls behind at pair boundaries
                        epi_q.popleft()()
                    p = state["cur"]
                    if not pend.get(p):
                        break
                    pi, ptA, ptB = pend[p].popleft()
                    state["outstanding"] -= 1
                    if pi == 0:
                        O_tiles[p] = opairp.tile(
                            [97, 512], f32, tag="opair", name="opair"
                        )
                    emit_pv(p, pi, ptA, ptB)
                    if pi == last_pi:
                        emit_epilogue(p)
                        state["cur"] += 1
                    if not force:
                        break

            PP = min(PRO_PAIRS, NP)
            next_p = [0] * NP
            for v in range(NT // 8):
                emit_chunk(v)
                if v == NT // 8 - 1:
                    nc.sync.dma_start(out=kva_sb[0 : D + 1, :], in_=kva_d)
                    nc.sync.dma_start(out=kva_sb[64 : 64 + D + 1, :], in_=kva_d)
                    nc.sync.dma_start(out=eye_sb, in_=eye_d)
                    for v2 in range(NT // 8):
                        nc.sync.dma_start(
                            out=xrows[:, 8 * v2 : 8 * (v2 + 1), :],
                            in_=xb_t[:, 8 * v2 : 8 * (v2 + 1), :],
                        )
                for pair in range(PP):
                    if pair > v:
                        continue  # this pair's query columns aren't loaded yet
                    while next_p[pair] < len(packs) and (
                        packs[next_p[pair]][0] + packs[next_p[pair]][1] - 1
                        <= 8 * v + 7
                    ):
                        pop_pv()
                        push_pair_pack(pair, next_p[pair])
                        next_p[pair] += 1
            for pair in range(NP):
                for pi in range(next_p[pair], len(packs)):
                    pop_pv()
                    push_pair_pack(pair, pi)
            pop_pv(force=True)
            while epi_q:
                epi_q.popleft()()
            loop_cm.__exit__(None, None, None)

    nc.compile()
    return nc


def _prep_weights(wf, uf, wg, ug, wh, uh, wv, uv, bh, bv, gamma, bg):
    kf = _sn_kernel_host(wf, uf)
    kg = _sn_kernel_host(wg, ug)
    kh = _sn_kernel_host(wh, uh)
    kv = _sn_kernel_host(wv, uv)
    gamma = float(np.asarray(gamma, np.float64)[0])
    bvp = np.asarray(bh, np.float64) @ np.asarray(kv, np.float64) + np.asarray(
        bv, np.float64
    )
    kva = np.concatenate(
        [gamma * np.asarray(kv, np.float64), (gamma * bvp)[None, :]], axis=0
    ).astype(np.float32)
    kf64 = np.asarray(kf, np.float64)
    kg64 = np.asarray(kg, np.float64)
    bg64 = np.asarray(bg, np.float64)
    bgf = (LOG2E * (kf64 @ bg64))[:, None]
    kfgf = np.concatenate([16.0 * kf64, bgf, np.zeros((C, 31))], axis=1).astype(
        np.float32
    )
    kfgg = np.concatenate(
        [8.0 * LOG2E * kg64, np.zeros((C, 32))], axis=1
    ).astype(np.float32)
    return kfgf, kfgg, kh, kva


def kernel(
    x, wf, bf, uf, wg, bg, ug, wh, bh, uh, wv, bv, uv, gamma, _n=None
) -> np.ndarray:
    global LAST_RESULTS, LAST_IN_MAPS
    from concourse import bass_utils

    n = _n or N_FULL
    if (n, 1) not in _BUILD_CACHE:
        _BUILD_CACHE[(n, 1)] = _build(n)
    nc = _BUILD_CACHE[(n, 1)]

    import ml_dtypes

    kfgf, kfgg, kh, kva = _prep_weights(
        wf, uf, wg, ug, wh, uh, wv, uv, bh, bv, gamma, bg
    )
    bfd = ml_dtypes.bfloat16
    kfgf = np.ascontiguousarray(kfgf.astype(np.float16))
    kfgg = np.ascontiguousarray(kfgg.astype(np.float16))
    kh = np.ascontiguousarray(kh.astype(np.float16))
    kva = np.ascontiguousarray(kva.astype(bfd))
    eye = np.ascontiguousarray(np.eye(128, dtype=np.float32).astype(bfd))
    cbias = np.zeros((128, 1), np.float32)
    cbias[32, 0] = 128.0  # g's const-128 row
    cbias[96, 0] = 128.0  # replica

    x = np.asarray(x, np.float32)
    b = x.shape[0]
    xflat = np.ascontiguousarray(x.reshape(b, -1, C)[:, :n, :])
    xflat_f16 = np.ascontiguousarray(xflat.astype(np.float16))
    in_maps = [
        {
            "xb": np.ascontiguousarray(xflat[i]),
            "xbh": xflat_f16[i],
            "kfgf": kfgf,
            "kfgg": kfgg,
            "kh": kh,
            "kva": kva,
            "eye": eye,
            "cbias": cbias,
        }
        for i in range(b)
    ]

    LAST_IN_MAPS = in_maps
    trace = bool(int(os.environ.get("BASS_KERNEL_TRACE", "0")))
    try:
        LAST_RESULTS = bass_utils.run_bass_kernel_spmd(
            nc,
            in_maps,
            core_ids=list(range(b)),
            trace=trace,
            trace_cores=[0] if trace else None,
        )
    except ModuleNotFoundError:
        LAST_RESULTS = bass_utils.run_bass_kernel_spmd(
            nc, in_maps, core_ids=list(range(b))
        )
    out = np.stack([r["y"] for r in LAST_RESULTS.results], axis=0)
    if n == N_FULL:
        out = out.reshape(b, H, W, C)
    return out
